# revision 2
# baseline (speedup 1.0000x reference)
"""Trainium2 Bass kernel for visual cross-attention:
    proj   = text @ W_w.T + W_b          [B,T,D]
    scores = proj @ local.T              [B,T,L]
    attn   = softmax(scores, axis=-1)
    out    = attn @ local                [B,T,D]

B=16, T=L=D=1024, fp32. Data-parallel over batch: 8 cores x 2 batches.
Score-path matmuls run as float32r (full PE rate at free>=256, ~1e-3 rel
err); the attention-weight path (attnT, lN, phase C) runs bf16 -- the
softmax output is in [0,1] so bf16 keeps ~3e-3 overall rel err, halves
the lN DMA, and lets the attn transposes run at 1.0 cycles/row.

Per core, per batch, per T-tile (512 t's):
  A: projT[e,t]   = W^T-chunks.T @ textT-chunks        (PE, accum over d)
  B: scores[t,l]  = projT-chunks.T @ localT-chunks     (PE, accum over e)
     softmax over l (free dim): DVE max, ACT exp(+bias,-max, accum sum)
  T: attnT[l,t]   = et-chunk.T @ diag(1/s)  -- ONE bf16 matmul per
     128x128 block fuses the transpose with the softmax 1/sum scale
     (diag = ident * recip(s), built on DVE). Emitted one q-chunk behind
     B so the next B covers softmax latency.
  C: outT[d,t]    = lN-chunks.T @ attnT-chunks         (PE bf16, accum l)

PE p-state: the tensor engine clocks 0.65/1.2 GHz until ~3us of
continuous execution, 2.4 GHz after. The DMA-bound startup would leave
it cold, so dummy bf16 matmuls (on a memset tile, into a scratch PSUM
slot) pad the gaps: a t=0 warmup train, fills between phase-A chunk
arrivals, and a bridge to the first phase B. The last tile interleaves
T/C per q-block so the output DMA starts ~10us earlier.
Host side only reshapes/transposes (layout prep + final [d,t]->[t,d]).
"""
import sys

sys.path.insert(0, "/opt/trn_rl_repo")
import numpy as np

B, T, L, D = 16, 1024, 1024, 1024
NCORES = 8
NB = B // NCORES          # batches per core
TT = 512                  # T-tile (moving dim for phases A/C)
NT = T // TT              # T-tiles per batch
NC8 = D // 128            # 128-chunks along d/e/l
NQ = TT // 128            # 128-t chunks per T-tile

# PE warmup / gap-filler dummy matmul counts (tuned on trace evidence)
W0_N = 64                 # t=0 warmup train
DA_N = 12                 # per-dc filler inside A(0,0) ec=0 trickle
W1_N = 40                 # bridge A(0,0) -> B(0,0)
DB_N = 10                 # per-ec filler inside B(0,0) q0 lh0 trickle

_cache = {}


def _build():
    import concourse.tile as tile
    from concourse import bacc, mybir
    from concourse.masks import make_identity

    f32 = mybir.dt.float32
    f32r = mybir.dt.float32r
    bf16 = mybir.dt.bfloat16
    Act = mybir.ActivationFunctionType

    nc = bacc.Bacc("TRN2", target_bir_lowering=False, debug=False,
                   num_devices=NCORES)
    tT_d = nc.dram_tensor("tT", [NB, D, T], f32r, kind="ExternalInput").ap()
    lT_d = nc.dram_tensor("lT", [NB, D, L], f32r, kind="ExternalInput").ap()
    lN_d = nc.dram_tensor("lN", [NB, L, D], bf16, kind="ExternalInput").ap()
    wT_d = nc.dram_tensor("wT", [D, D], f32r, kind="ExternalInput").ap()
    wb_d = nc.dram_tensor("wb", [128, NC8], f32, kind="ExternalInput").ap()
    outT_d = nc.dram_tensor("outT", [NB, D, T], f32, kind="ExternalOutput").ap()

    with tile.TileContext(nc) as tc:
        with tc.tile_pool(name="const", bufs=1) as constp, \
             tc.tile_pool(name="res", bufs=1) as resp, \
             tc.tile_pool(name="work", bufs=2) as workp, \
             tc.tile_pool(name="proj", bufs=3) as projp, \
             tc.tile_pool(name="single", bufs=1) as singlep, \
             tc.tile_pool(name="stats", bufs=8) as statsp, \
             tc.tile_pool(name="psS", bufs=2, space="PSUM") as psS_p, \
             tc.tile_pool(name="psMM", bufs=2, space="PSUM") as psMM_p, \
             tc.tile_pool(name="psT", bufs=2, space="PSUM") as psT_p:

            # round-robin loads across all 3 DMA-capable queues (sync/scalar
            # HWDGE + gpsimd SWDGE), in consumption order; each queue peaks
            # ~110-130GB/s, together ~350GB/s (HBM-bound).
            # The scalar engine is a DMA-issue engine (qAct HWDGE) AND the
            # softmax/copy engine. A long load backlog on it blocks ACT
            # compute behind DMA sem-pool wrap waits. So: the startup-
            # critical prefix (wt, tT(0,*), lT(0)) round-robins over all 3
            # queues for max bandwidth (scalar kept <= 8 issues, its sem
            # pool); everything later avoids scalar.
            queues = [[nc.sync, nc.scalar, nc.gpsimd]]
            qi = [0]

            def load(out, in_):
                qs = queues[0]
                qs[qi[0] % len(qs)].dma_start(out=out, in_=in_)
                qi[0] += 1

            def load_tT(b, it):
                t0 = it * TT
                tile_ = workp.tile([128, NC8, TT], f32r, tag="tT")
                for dc in range(NC8):
                    load(tile_[:, dc, :],
                         tT_d[b, dc * 128:(dc + 1) * 128, t0:t0 + TT])
                return tile_

            # dummy-matmul scratch: memset SBUF tile + PSUM slots cycled
            # from the psT pool. Dummies have no data deps, so the in-order
            # tensor queue runs them while real matmuls wait on DMA sems --
            # keeping the PE clock ramped. Each block writes one scratch
            # tile (WAW on the same engine needs no semaphores).
            dummy_sb = constp.tile([128, 128], bf16, tag="dummy_sb")
            nc.vector.memset(dummy_sb[:], 0.0)

            def dummies(n):
                dps = psT_p.tile([128, 128], f32, tag="tp")
                for _ in range(n):
                    nc.tensor.matmul(dps[:], dummy_sb[:], dummy_sb[:],
                                     start=True, stop=True)

            dummies(W0_N)

            wt_sb = constp.tile([128, NC8, D], f32r, tag="wt")
            wb_sb = constp.tile([128, NC8], f32, tag="wb")
            tT_first = workp.tile([128, NC8, TT], f32r, tag="tT")
            # the first matmul needs only wt[:, 0, 0:128] + tT(0,0) chunk 0:
            # both go on sync, the earliest-spinning DMA queue
            nc.sync.dma_start(out=wt_sb[:, 0, 0:128], in_=wT_d[0:128, 0:128])
            nc.sync.dma_start(out=tT_first[:, 0, :], in_=tT_d[0, 0:128, 0:TT])
            qi[0] = 2  # keep round-robin balanced past the two sync loads
            load(wt_sb[:, 0, 128:D], wT_d[0:128, 128:D])
            for dc in range(1, NC8):
                load(wt_sb[:, dc, :], wT_d[dc * 128:(dc + 1) * 128, :])
                load(tT_first[:, dc, :], tT_d[0, dc * 128:(dc + 1) * 128, 0:TT])
            load(wb_sb[:], wb_d[:])
            # scalar has now issued <=8 DMAs (its sem pool) -- no wrap waits.
            # Free it for ACT compute; all later DMA goes to sync+gpsimd.
            queues[0] = [nc.sync, nc.gpsimd]
            qi[0] = 0

            lT_tiles = {}
            lN_tiles = {}

            def load_locals(b):
                # lT keeps full 4KB-row DMAs (f32); lN rides bf16 (2KB rows,
                # half the bytes) since phase C runs bf16 anyway
                lT_sb = resp.tile([128, NC8, L], f32r, tag="lT")
                lN_sb = resp.tile([128, NC8, D], bf16, tag="lN")
                for c in range(NC8):
                    load(lT_sb[:, c, :], lT_d[b, c * 128:(c + 1) * 128, :])
                for c in range(NC8):
                    load(lN_sb[:, c, :], lN_d[b, c * 128:(c + 1) * 128, :])
                lT_tiles[b] = lT_sb
                lN_tiles[b] = lN_sb

            # startup DMA order: wt+tT(0,0) [6MB] -> lT(0) [4MB, gates the
            # first softmax] -> tT(0,1) [2MB, phase A prefetch during
            # B(0,0)] -> lN(0) [2MB bf16, gates phase C ~50us in]
            lT_b0s = resp.tile([128, NC8, L], f32r, tag="lT")
            for c in range(NC8):
                load(lT_b0s[:, c, :], lT_d[0, c * 128:(c + 1) * 128, :])
            lT_tiles[0] = lT_b0s
            tT_b01 = load_tT(0, 1)
            lN_b0 = resp.tile([128, NC8, D], bf16, tag="lN")
            for c in range(NC8):
                load(lN_b0[:, c, :], lN_d[0, c * 128:(c + 1) * 128, :])
            lN_tiles[0] = lN_b0
            # identity for the diag(1/s) transpose trick -- not needed until
            # ~35us; built after the startup loads so gpsimd's DMA queue
            # isn't delayed
            identf = constp.tile([128, 128], f32, tag="identf")
            make_identity(nc, identf[:])
            ident_b = constp.tile([128, 128], bf16, tag="ident")
            nc.vector.tensor_copy(ident_b[:], identf[:])

            def phase_a(tT_sb, trickle=False):
                projT = projp.tile([128, NC8, TT], f32r, tag="projT")
                for ec in range(NC8):
                    psA = psMM_p.tile([128, TT], f32, tag="mm")
                    for dc in range(NC8):
                        nc.tensor.matmul(
                            psA[:],
                            wt_sb[:, dc, ec * 128:(ec + 1) * 128],
                            tT_sb[:, dc, :],
                            start=(dc == 0), stop=(dc == NC8 - 1))
                        # A(0,0) ec=0 consumes (wt,tT) chunk pairs as they
                        # land ~2.1us apart; filler keeps the clock up
                        if trickle and ec == 0 and dc < NC8 - 1:
                            dummies(DA_N)
                    nc.scalar.activation(projT[:, ec, :], psA[:], Act.Identity,
                                         bias=wb_sb[:, ec:ec + 1], scale=1.0)
                return projT

            def transposes(attnT, et, q, diag):
                # attnT[l,t] = et.T @ diag(1/s): one bf16 matmul per 128x128
                # block transposes AND applies the softmax scale
                for lq in range(NC8):
                    psT = psT_p.tile([128, 128], f32, tag="tp")
                    nc.tensor.matmul(psT[:],
                                     et[:, lq * 128:(lq + 1) * 128],
                                     diag[:], start=True, stop=True)
                    dst = attnT[:, lq, q * 128:(q + 1) * 128]
                    if lq % 2 == 0:
                        nc.vector.tensor_copy(dst, psT[:])
                    else:
                        nc.scalar.copy(dst, psT[:])

            projTs = {(0, 0): phase_a(tT_first, trickle=True)}
            preloaded = {(0, 1): tT_b01}
            dummies(W1_N)  # bridge: A(0,0) done ~22us, lT(0) lands ~31us

            tiles = [(b, it) for b in range(NB) for it in range(NT)]
            for i, (b, it) in enumerate(tiles):
                t0 = it * TT
                last = i == len(tiles) - 1
                if b > 0 and it == 0:
                    load_locals(b)
                projT = projTs[(b, it)]
                lT_sb, lN_sb = lT_tiles[b], lN_tiles[b]
                # ---- phase B + softmax; transposes one q behind; the last
                # tile also interleaves phase C per q so out-DMA starts early
                attnT = singlep.tile([128, NC8, TT], bf16, tag="attnT")
                outcps = {}

                def phase_c_q(q):
                    # last-tile phase C for one t-block: free dim 128
                    for dc in range(NC8):
                        psC = psMM_p.tile([128, 128], f32, tag="mm")
                        for lq in range(NC8):
                            nc.tensor.matmul(
                                psC[:],
                                lN_sb[:, lq, dc * 128:(dc + 1) * 128],
                                attnT[:, lq, q * 128:(q + 1) * 128],
                                start=(lq == 0), stop=(lq == NC8 - 1))
                        oc = workp.tile([128, 128], f32, tag="outcq",
                                        bufs=4)
                        if dc % 2 == 0:
                            nc.vector.tensor_copy(oc[:], psC[:])
                        else:
                            nc.scalar.copy(oc[:], psC[:])
                        sq = [nc.sync, nc.gpsimd, nc.scalar][(q * NC8 + dc) % 3]
                        sq.dma_start(
                            out=outT_d[b, dc * 128:(dc + 1) * 128,
                                       t0 + q * 128:t0 + (q + 1) * 128],
                            in_=oc[:])

                pending = None
                for q in range(NQ):
                    psS = psS_p.tile([128, L], f32, tag="scores")
                    for lh in range(L // 512):
                        for ec in range(NC8):
                            nc.tensor.matmul(
                                psS[:, lh * 512:(lh + 1) * 512],
                                projT[:, ec, q * 128:(q + 1) * 128],
                                lT_sb[:, ec, lh * 512:(lh + 1) * 512],
                                start=(ec == 0), stop=(ec == NC8 - 1))
                            # B(0,0) q0 lh0 consumes lT(0) chunks as they
                            # land; filler keeps the clock up
                            if i == 0 and q == 0 and lh == 0 and ec < NC8 - 1:
                                dummies(DB_N)
                    nm = statsp.tile([128, 1], f32, tag="nm")
                    nc.vector.tensor_reduce(nm[:], psS[:],
                                            axis=mybir.AxisListType.X,
                                            op=mybir.AluOpType.max,
                                            negate=True)
                    et = workp.tile([128, L], bf16, tag="et")
                    s = statsp.tile([128, 1], f32, tag="s")
                    nc.scalar.activation(et[:], psS[:], Act.Exp,
                                         bias=nm[:, 0:1], scale=1.0,
                                         accum_out=s[:])
                    rr = statsp.tile([128, 1], f32, tag="rr")
                    nc.vector.reciprocal(rr[:], s[:])
                    diag = statsp.tile([128, 128], bf16, tag="diag", bufs=2)
                    nc.vector.tensor_scalar_mul(diag[:], ident_b[:],
                                                rr[:, 0:1])
                    if pending is not None:
                        transposes(attnT, *pending)
                        if last and pending[1] >= 1:
                            phase_c_q(pending[1] - 1)
                    pending = (et, q, diag)
                # prefetch the next tile's A phase here: its matmuls fill
                # the exp(q3)->transpose latency bubble and the batch
                # boundary, instead of the PE idling on them
                if i + 1 < len(tiles):
                    nb_, nit_ = tiles[i + 1]
                    if (nb_, nit_) not in projTs:
                        tT_next = preloaded.pop((nb_, nit_), None)
                        if tT_next is None:
                            tT_next = load_tT(nb_, nit_)
                        projTs[(nb_, nit_)] = phase_a(tT_next)
                transposes(attnT, *pending)
                if last:
                    phase_c_q(NQ - 2)
                    phase_c_q(NQ - 1)
                    continue
                # ---- phase C (monolithic, free dim 512): outT[d, t] ----
                for dc in range(NC8):
                    psC = psMM_p.tile([128, TT], f32, tag="mm")
                    for lq in range(NC8):
                        nc.tensor.matmul(
                            psC[:],
                            lN_sb[:, lq, dc * 128:(dc + 1) * 128],
                            attnT[:, lq, :],
                            start=(lq == 0), stop=(lq == NC8 - 1))
                    outcp = workp.tile([128, TT], f32, tag="outcp")
                    if dc % 2 == 0:
                        nc.vector.tensor_copy(outcp[:], psC[:])
                    else:
                        nc.scalar.copy(outcp[:], psC[:])
                    sq = queues[0][dc % 2]
                    sq.dma_start(
                        out=outT_d[b, dc * 128:(dc + 1) * 128, t0:t0 + TT],
                        in_=outcp[:])
    nc.compile()
    return nc


def _get_nc():
    if "nc" not in _cache:
        _cache["nc"] = _build()
    return _cache["nc"]


def _prep_inputs(text_features, local_features, W_w, W_b):
    import ml_dtypes

    text = np.asarray(text_features, dtype=np.float32)
    local = np.asarray(local_features, dtype=np.float32)
    W = np.asarray(W_w, dtype=np.float32)
    bvec = np.asarray(W_b, dtype=np.float32)

    wT = np.ascontiguousarray(W.T)                       # [d, e]
    wb = np.ascontiguousarray(bvec.reshape(NC8, 128).T)  # [128, ec]
    in_maps = []
    for c in range(NCORES):
        sl = slice(c * NB, (c + 1) * NB)
        in_maps.append({
            "tT": np.ascontiguousarray(text[sl].transpose(0, 2, 1)),
            "lT": np.ascontiguousarray(local[sl].transpose(0, 2, 1)),
            "lN": np.ascontiguousarray(local[sl].astype(ml_dtypes.bfloat16)),
            "wT": wT,
            "wb": wb,
        })
    return in_maps


def _run(inputs, trace=False):
    from concourse.bass_utils import run_bass_kernel_spmd

    nc = _get_nc()
    in_maps = _prep_inputs(**inputs)
    res = run_bass_kernel_spmd(nc, in_maps, list(range(NCORES)), trace=trace)
    out = np.empty((B, T, D), dtype=np.float32)
    for c in range(NCORES):
        outT = res.results[c]["outT"]                    # [NB, d, t]
        out[c * NB:(c + 1) * NB] = outT.transpose(0, 2, 1)
    return out, res


def kernel(**inputs):
    out, _ = _run(inputs, trace=False)
    return out


# revision 8
# speedup vs baseline: 1.0186x; 1.0186x over previous
"""Trainium2 Bass kernel for visual cross-attention:
    proj   = text @ W_w.T + W_b          [B,T,D]
    scores = proj @ local.T              [B,T,L]
    attn   = softmax(scores, axis=-1)
    out    = attn @ local                [B,T,D]

B=16, T=L=D=1024, fp32. Data-parallel over batch: 8 cores x 2 batches.
Score-path matmuls run as float32r (full PE rate at free>=256, ~1e-3 rel
err); the attention-weight path (attnT, lN, phase C) runs bf16 -- the
softmax output is in [0,1] so bf16 keeps ~3e-3 overall rel err, halves
the lN DMA, and lets the attn transposes run at 1.0 cycles/row.

Per core, per batch, per T-tile (512 t's):
  A: projT[e,t]   = W^T-chunks.T @ textT-chunks        (PE, accum over d)
  B: scores[t,l]  = projT-chunks.T @ localT-chunks     (PE, accum over e)
     softmax over l (free dim): DVE max, ACT exp(+bias,-max, accum sum)
  T: attnT[l,t]   = et-chunk.T @ diag(1/s)  -- ONE bf16 matmul per
     128x128 block fuses the transpose with the softmax 1/sum scale
     (diag = ident * recip(s), built on DVE). Emitted one q-chunk behind
     B so the next B covers softmax latency.
  C: outT[d,t]    = lN-chunks.T @ attnT-chunks         (PE bf16, accum l)

PE p-state: the tensor engine clocks 0.65/1.2 GHz until ~3us of
continuous execution, 2.4 GHz after. The DMA-bound startup would leave
it cold, so dummy bf16 matmuls (on a memset tile, into a scratch PSUM
slot) pad the gaps: a t=0 warmup train, fills between phase-A chunk
arrivals, and a bridge to the first phase B. The last tile interleaves
T/C per q-block so the output DMA starts ~10us earlier.
Host side only reshapes/transposes (layout prep + final [d,t]->[t,d]).
"""
import sys

sys.path.insert(0, "/opt/trn_rl_repo")
import numpy as np

B, T, L, D = 16, 1024, 1024, 1024
NCORES = 8
NB = B // NCORES          # batches per core
TT = 512                  # T-tile (moving dim for phases A/C)
NT = T // TT              # T-tiles per batch
NC8 = D // 128            # 128-chunks along d/e/l
NQ = TT // 128            # 128-t chunks per T-tile

# PE warmup / gap-filler dummy matmul counts (tuned on trace evidence)
W0_N = 64                 # t=0 warmup train
DA_N = 12                 # per-dc filler inside A(0,0) ec=0 trickle
W1_N = 40                 # bridge A(0,0) -> B(0,0)
DB_N = 10                 # per-ec filler inside B(0,0) q0 lh0 trickle

_cache = {}


def _build():
    import concourse.tile as tile
    from concourse import bacc, mybir
    from concourse.masks import make_identity

    f32 = mybir.dt.float32
    f32r = mybir.dt.float32r
    bf16 = mybir.dt.bfloat16
    Act = mybir.ActivationFunctionType

    nc = bacc.Bacc("TRN2", target_bir_lowering=False, debug=False,
                   num_devices=NCORES)
    tT_d = nc.dram_tensor("tT", [NB, D, T], f32r, kind="ExternalInput").ap()
    lT_d = nc.dram_tensor("lT", [NB, D, L], f32r, kind="ExternalInput").ap()
    lN_d = nc.dram_tensor("lN", [NB, L, D], bf16, kind="ExternalInput").ap()
    wT_d = nc.dram_tensor("wT", [D, D], f32r, kind="ExternalInput").ap()
    wb_d = nc.dram_tensor("wb", [128, NC8], f32, kind="ExternalInput").ap()
    outT_d = nc.dram_tensor("outT", [NB, D, T], f32, kind="ExternalOutput").ap()

    with tile.TileContext(nc) as tc:
        with tc.tile_pool(name="const", bufs=1) as constp, \
             tc.tile_pool(name="res", bufs=1) as resp, \
             tc.tile_pool(name="work", bufs=2) as workp, \
             tc.tile_pool(name="proj", bufs=3) as projp, \
             tc.tile_pool(name="single", bufs=1) as singlep, \
             tc.tile_pool(name="stats", bufs=8) as statsp, \
             tc.tile_pool(name="psS", bufs=2, space="PSUM") as psS_p, \
             tc.tile_pool(name="psMM", bufs=2, space="PSUM") as psMM_p, \
             tc.tile_pool(name="psT", bufs=2, space="PSUM") as psT_p:

            # round-robin loads across all 3 DMA-capable queues (sync/scalar
            # HWDGE + gpsimd SWDGE), in consumption order; each queue peaks
            # ~110-130GB/s, together ~350GB/s (HBM-bound).
            # The scalar engine is a DMA-issue engine (qAct HWDGE) AND the
            # softmax/copy engine. A long load backlog on it blocks ACT
            # compute behind DMA sem-pool wrap waits. So: the startup-
            # critical prefix (wt, tT(0,*), lT(0)) round-robins over all 3
            # queues for max bandwidth (scalar kept <= 8 issues, its sem
            # pool); everything later avoids scalar.
            queues = [[nc.sync, nc.scalar, nc.gpsimd]]
            qi = [0]

            def load(out, in_):
                qs = queues[0]
                qs[qi[0] % len(qs)].dma_start(out=out, in_=in_)
                qi[0] += 1

            def load_tT(b, it):
                t0 = it * TT
                tile_ = workp.tile([128, NC8, TT], f32r, tag="tT")
                for dc in range(NC8):
                    load(tile_[:, dc, :],
                         tT_d[b, dc * 128:(dc + 1) * 128, t0:t0 + TT])
                return tile_

            # dummy-matmul scratch: memset SBUF tile + PSUM slots cycled
            # from the psT pool. Dummies have no data deps, so the in-order
            # tensor queue runs them while real matmuls wait on DMA sems --
            # keeping the PE clock ramped. Each block writes one scratch
            # tile (WAW on the same engine needs no semaphores).
            dummy_sb = constp.tile([128, 128], bf16, tag="dummy_sb")
            nc.vector.memset(dummy_sb[:], 0.0)

            def dummies(n):
                dps = psT_p.tile([128, 128], f32, tag="tp")
                for _ in range(n):
                    nc.tensor.matmul(dps[:], dummy_sb[:], dummy_sb[:],
                                     start=True, stop=True)

            dummies(W0_N)

            wt_sb = constp.tile([128, NC8, D], f32r, tag="wt")
            wb_sb = constp.tile([128, NC8], f32, tag="wb")
            tT_first = workp.tile([128, NC8, TT], f32r, tag="tT")
            # the first matmul needs only wt[:, 0, 0:128] + tT(0,0) chunk 0:
            # both go on sync, the earliest-spinning DMA queue
            nc.sync.dma_start(out=wt_sb[:, 0, 0:128], in_=wT_d[0:128, 0:128])
            nc.sync.dma_start(out=tT_first[:, 0, :], in_=tT_d[0, 0:128, 0:TT])
            qi[0] = 2  # keep round-robin balanced past the two sync loads
            load(wt_sb[:, 0, 128:D], wT_d[0:128, 128:D])
            for dc in range(1, NC8):
                load(wt_sb[:, dc, :], wT_d[dc * 128:(dc + 1) * 128, :])
                load(tT_first[:, dc, :], tT_d[0, dc * 128:(dc + 1) * 128, 0:TT])
            load(wb_sb[:], wb_d[:])
            # scalar has now issued <=8 DMAs (its sem pool) -- no wrap waits.
            # Free it for ACT compute; all later DMA goes to sync+gpsimd.
            queues[0] = [nc.sync, nc.gpsimd]
            qi[0] = 0

            lT_tiles = {}
            lN_tiles = {}

            def load_locals(b):
                # lT keeps full 4KB-row DMAs (f32); lN rides bf16 (2KB rows,
                # half the bytes) since phase C runs bf16 anyway
                lT_sb = resp.tile([128, NC8, L], f32r, tag="lT")
                lN_sb = resp.tile([128, NC8, D], bf16, tag="lN")
                for c in range(NC8):
                    load(lT_sb[:, c, :], lT_d[b, c * 128:(c + 1) * 128, :])
                for c in range(NC8):
                    load(lN_sb[:, c, :], lN_d[b, c * 128:(c + 1) * 128, :])
                lT_tiles[b] = lT_sb
                lN_tiles[b] = lN_sb

            # startup DMA order: wt+tT(0,0) [6MB] -> lT(0) [4MB, gates the
            # first softmax] -> lN(0) [2MB bf16, gates phase C] -> tT(0,1).
            # Each engine's DMA sem pool is 8 deep: issue #k stalls until
            # issue #k-8's sem retires at its first consumer. lN(0) must sit
            # <=8 issues after early-retiring sems (startup/lT0), NOT after
            # tT(0,1) whose consumer A(0,1) is deferred to the prefetch slot.
            lT_b0s = resp.tile([128, NC8, L], f32r, tag="lT")
            for c in range(NC8):
                load(lT_b0s[:, c, :], lT_d[0, c * 128:(c + 1) * 128, :])
            lT_tiles[0] = lT_b0s
            lN_b0 = resp.tile([128, NC8, D], bf16, tag="lN")
            for c in range(NC8):
                load(lN_b0[:, c, :], lN_d[0, c * 128:(c + 1) * 128, :])
            lN_tiles[0] = lN_b0
            tT_b01 = load_tT(0, 1)
            # identity for the diag(1/s) transpose trick -- not needed until
            # ~35us; built after the startup loads so gpsimd's DMA queue
            # isn't delayed
            identf = constp.tile([128, 128], f32, tag="identf")
            make_identity(nc, identf[:])
            ident_b = constp.tile([128, 128], bf16, tag="ident")
            nc.vector.tensor_copy(ident_b[:], identf[:])

            def phase_a(tT_sb, trickle=False):
                projT = projp.tile([128, NC8, TT], f32r, tag="projT")
                for ec in range(NC8):
                    psA = psMM_p.tile([128, TT], f32, tag="mm")
                    for dc in range(NC8):
                        nc.tensor.matmul(
                            psA[:],
                            wt_sb[:, dc, ec * 128:(ec + 1) * 128],
                            tT_sb[:, dc, :],
                            start=(dc == 0), stop=(dc == NC8 - 1))
                        # A(0,0) ec=0 consumes (wt,tT) chunk pairs as they
                        # land ~2.1us apart; filler keeps the clock up
                        if trickle and ec == 0 and dc < NC8 - 1:
                            dummies(DA_N)
                    nc.scalar.activation(projT[:, ec, :], psA[:], Act.Identity,
                                         bias=wb_sb[:, ec:ec + 1], scale=1.0)
                return projT

            def transposes(attnT, et, q, diag):
                # attnT[l,t] = et.T @ diag(1/s): one bf16 matmul per 128x128
                # block transposes AND applies the softmax scale
                for lq in range(NC8):
                    psT = psT_p.tile([128, 128], f32, tag="tp")
                    nc.tensor.matmul(psT[:],
                                     et[:, lq * 128:(lq + 1) * 128],
                                     diag[:], start=True, stop=True)
                    dst = attnT[:, lq, q * 128:(q + 1) * 128]
                    if lq % 2 == 0:
                        nc.vector.tensor_copy(dst, psT[:])
                    else:
                        nc.scalar.copy(dst, psT[:])

            projTs = {(0, 0): phase_a(tT_first, trickle=True)}
            preloaded = {(0, 1): tT_b01}
            dummies(W1_N)  # bridge: A(0,0) done ~22us, lT(0) lands ~31us

            tiles = [(b, it) for b in range(NB) for it in range(NT)]
            for i, (b, it) in enumerate(tiles):
                t0 = it * TT
                last = i == len(tiles) - 1
                if b > 0 and it == 0:
                    load_locals(b)
                projT = projTs[(b, it)]
                lT_sb, lN_sb = lT_tiles[b], lN_tiles[b]
                # ---- phase B + softmax; transposes one q behind; the last
                # tile also interleaves phase C per q so out-DMA starts early
                attnT = singlep.tile([128, NC8, TT], bf16, tag="attnT")

                def phase_c_half(h):
                    # last-tile phase C for a 256-t half: free dim 256 keeps
                    # the 97ns LDWEIGHTS hidden (128-free matmuls can't),
                    # while letting out-DMA start after T(q1) / T(q3)
                    for dc in range(NC8):
                        psC = psMM_p.tile([128, 256], f32, tag="mm")
                        for lq in range(NC8):
                            nc.tensor.matmul(
                                psC[:],
                                lN_sb[:, lq, dc * 128:(dc + 1) * 128],
                                attnT[:, lq, h * 256:(h + 1) * 256],
                                start=(lq == 0), stop=(lq == NC8 - 1))
                        oc = workp.tile([128, 256], f32, tag="outcq",
                                        bufs=4)
                        if dc % 2 == 0:
                            nc.vector.tensor_copy(oc[:], psC[:])
                        else:
                            nc.scalar.copy(oc[:], psC[:])
                        sq = [nc.sync, nc.gpsimd, nc.scalar][(h * NC8 + dc) % 3]
                        sq.dma_start(
                            out=outT_d[b, dc * 128:(dc + 1) * 128,
                                       t0 + h * 256:t0 + (h + 1) * 256],
                            in_=oc[:])

                pending = None
                for q in range(NQ):
                    psS = psS_p.tile([128, L], f32, tag="scores")
                    for lh in range(L // 512):
                        for ec in range(NC8):
                            nc.tensor.matmul(
                                psS[:, lh * 512:(lh + 1) * 512],
                                projT[:, ec, q * 128:(q + 1) * 128],
                                lT_sb[:, ec, lh * 512:(lh + 1) * 512],
                                start=(ec == 0), stop=(ec == NC8 - 1))
                            # B(0,0) q0 lh0 consumes lT(0) chunks as they
                            # land; filler keeps the clock up
                            if i == 0 and q == 0 and lh == 0 and ec < NC8 - 1:
                                dummies(DB_N)
                    nm = statsp.tile([128, 1], f32, tag="nm")
                    nc.vector.tensor_reduce(nm[:], psS[:],
                                            axis=mybir.AxisListType.X,
                                            op=mybir.AluOpType.max,
                                            negate=True)
                    et = workp.tile([128, L], bf16, tag="et")
                    s = statsp.tile([128, 1], f32, tag="s")
                    nc.scalar.activation(et[:], psS[:], Act.Exp,
                                         bias=nm[:, 0:1], scale=1.0,
                                         accum_out=s[:])
                    rr = statsp.tile([128, 1], f32, tag="rr")
                    nc.vector.reciprocal(rr[:], s[:])
                    diag = statsp.tile([128, 128], bf16, tag="diag", bufs=2)
                    nc.vector.tensor_scalar_mul(diag[:], ident_b[:],
                                                rr[:, 0:1])
                    if pending is not None:
                        transposes(attnT, *pending)
                        if last and pending[1] == 1:
                            phase_c_half(0)
                    pending = (et, q, diag)
                # prefetch the next tile's A phase here: its matmuls fill
                # the exp(q3)->transpose latency bubble and the batch
                # boundary, instead of the PE idling on them
                if i + 1 < len(tiles):
                    nb_, nit_ = tiles[i + 1]
                    if (nb_, nit_) not in projTs:
                        tT_next = preloaded.pop((nb_, nit_), None)
                        if tT_next is None:
                            tT_next = load_tT(nb_, nit_)
                        projTs[(nb_, nit_)] = phase_a(tT_next)
                transposes(attnT, *pending)
                if last:
                    phase_c_half(1)
                    continue
                # ---- phase C (monolithic, free dim 512): outT[d, t] ----
                for dc in range(NC8):
                    psC = psMM_p.tile([128, TT], f32, tag="mm")
                    for lq in range(NC8):
                        nc.tensor.matmul(
                            psC[:],
                            lN_sb[:, lq, dc * 128:(dc + 1) * 128],
                            attnT[:, lq, :],
                            start=(lq == 0), stop=(lq == NC8 - 1))
                    outcp = workp.tile([128, TT], f32, tag="outcp")
                    if dc % 2 == 0:
                        nc.vector.tensor_copy(outcp[:], psC[:])
                    else:
                        nc.scalar.copy(outcp[:], psC[:])
                    sq = queues[0][dc % 2]
                    sq.dma_start(
                        out=outT_d[b, dc * 128:(dc + 1) * 128, t0:t0 + TT],
                        in_=outcp[:])
    nc.compile()
    return nc


def _get_nc():
    if "nc" not in _cache:
        _cache["nc"] = _build()
    return _cache["nc"]


def _prep_inputs(text_features, local_features, W_w, W_b):
    import ml_dtypes

    text = np.asarray(text_features, dtype=np.float32)
    local = np.asarray(local_features, dtype=np.float32)
    W = np.asarray(W_w, dtype=np.float32)
    bvec = np.asarray(W_b, dtype=np.float32)

    wT = np.ascontiguousarray(W.T)                       # [d, e]
    wb = np.ascontiguousarray(bvec.reshape(NC8, 128).T)  # [128, ec]
    in_maps = []
    for c in range(NCORES):
        sl = slice(c * NB, (c + 1) * NB)
        in_maps.append({
            "tT": np.ascontiguousarray(text[sl].transpose(0, 2, 1)),
            "lT": np.ascontiguousarray(local[sl].transpose(0, 2, 1)),
            "lN": np.ascontiguousarray(local[sl].astype(ml_dtypes.bfloat16)),
            "wT": wT,
            "wb": wb,
        })
    return in_maps


def _run(inputs, trace=False):
    from concourse.bass_utils import run_bass_kernel_spmd

    nc = _get_nc()
    in_maps = _prep_inputs(**inputs)
    res = run_bass_kernel_spmd(nc, in_maps, list(range(NCORES)), trace=trace)
    out = np.empty((B, T, D), dtype=np.float32)
    for c in range(NCORES):
        outT = res.results[c]["outT"]                    # [NB, d, t]
        out[c * NB:(c + 1) * NB] = outT.transpose(0, 2, 1)
    return out, res


def kernel(**inputs):
    out, _ = _run(inputs, trace=False)
    return out


# revision 14
# speedup vs baseline: 1.0288x; 1.0100x over previous
"""Trainium2 Bass kernel for visual cross-attention:
    proj   = text @ W_w.T + W_b          [B,T,D]
    scores = proj @ local.T              [B,T,L]
    attn   = softmax(scores, axis=-1)
    out    = attn @ local                [B,T,D]

B=16, T=L=D=1024, fp32. Data-parallel over batch: 8 cores x 2 batches.
Score-path matmuls run as float32r (full PE rate at free>=256, ~1e-3 rel
err); the attention-weight path (attnT, lN, phase C) runs bf16 -- the
softmax output is in [0,1] so bf16 keeps ~3e-3 overall rel err, halves
the lN DMA, and lets the attn transposes run at 1.0 cycles/row.

Per core, per batch, per T-tile (512 t's):
  A: projT[e,t]   = W^T-chunks.T @ textT-chunks        (PE, accum over d)
  B: scores[t,l]  = projT-chunks.T @ localT-chunks     (PE, accum over e)
     softmax over l (free dim): DVE max, ACT exp(+bias,-max, accum sum)
  T: attnT[l,t]   = et-chunk.T @ diag(1/s)  -- ONE bf16 matmul per
     128x128 block fuses the transpose with the softmax 1/sum scale
     (diag = ident * recip(s), built on DVE). Emitted one q-chunk behind
     B so the next B covers softmax latency.
  C: outT[d,t]    = lN-chunks.T @ attnT-chunks         (PE bf16, accum l)

PE p-state: the tensor engine clocks 0.65/1.2 GHz until ~3us of
continuous execution, 2.4 GHz after. The DMA-bound startup would leave
it cold, so dummy bf16 matmuls (on a memset tile, into a scratch PSUM
slot) pad the gaps: a t=0 warmup train, fills between phase-A chunk
arrivals, and a bridge to the first phase B. The last tile interleaves
T/C per q-block so the output DMA starts ~10us earlier.
Host side only reshapes/transposes (layout prep + final [d,t]->[t,d]).
"""
import sys

sys.path.insert(0, "/opt/trn_rl_repo")
import numpy as np

B, T, L, D = 16, 1024, 1024, 1024
NCORES = 8
NB = B // NCORES          # batches per core
TT = 512                  # T-tile (moving dim for phases A/C)
NT = T // TT              # T-tiles per batch
NC8 = D // 128            # 128-chunks along d/e/l
NQ = TT // 128            # 128-t chunks per T-tile

# PE warmup / gap-filler dummy matmul counts (tuned on trace evidence)
W0_N = 64                 # t=0 warmup train
DA_N = 20                 # per-dc filler inside A(0,0) ec=0 trickle
W1_N = 40                 # bridge A(0,0) -> B(0,0)
DB_N = 10                 # per-ec filler inside B(0,0) q0 lh0 trickle
PAD_N = 16                # sem-pool pad loads after the startup prefix

_cache = {}


def _build():
    import concourse.tile as tile
    from concourse import bacc, mybir
    from concourse.masks import make_identity

    f32 = mybir.dt.float32
    f32r = mybir.dt.float32r
    bf16 = mybir.dt.bfloat16
    Act = mybir.ActivationFunctionType

    nc = bacc.Bacc("TRN2", target_bir_lowering=False, debug=False,
                   num_devices=NCORES)
    tT_d = nc.dram_tensor("tT", [NB, D, T], f32r, kind="ExternalInput").ap()
    lT_d = nc.dram_tensor("lT", [NB, D, L], f32r, kind="ExternalInput").ap()
    lN_d = nc.dram_tensor("lN", [NB, L, D], bf16, kind="ExternalInput").ap()
    wT_d = nc.dram_tensor("wT", [D, D], f32r, kind="ExternalInput").ap()
    wb_d = nc.dram_tensor("wb", [128, NC8], f32, kind="ExternalInput").ap()
    outT_d = nc.dram_tensor("outT", [NB, D, T], f32, kind="ExternalOutput").ap()

    with tile.TileContext(nc) as tc:
        with tc.tile_pool(name="const", bufs=1) as constp, \
             tc.tile_pool(name="res", bufs=1) as resp, \
             tc.tile_pool(name="work", bufs=2) as workp, \
             tc.tile_pool(name="proj", bufs=3) as projp, \
             tc.tile_pool(name="single", bufs=1) as singlep, \
             tc.tile_pool(name="stats", bufs=8) as statsp, \
             tc.tile_pool(name="psS", bufs=2, space="PSUM") as psS_p, \
             tc.tile_pool(name="psMM", bufs=2, space="PSUM") as psMM_p, \
             tc.tile_pool(name="psT", bufs=2, space="PSUM") as psT_p:

            # round-robin loads across all 3 DMA-capable queues (sync/scalar
            # HWDGE + gpsimd SWDGE), in consumption order; each queue peaks
            # ~110-130GB/s, together ~350GB/s (HBM-bound).
            # The scalar engine is a DMA-issue engine (qAct HWDGE) AND the
            # softmax/copy engine. A long load backlog on it blocks ACT
            # compute behind DMA sem-pool wrap waits. So: the startup-
            # critical prefix (wt, tT(0,*), lT(0)) round-robins over all 3
            # queues for max bandwidth (scalar kept <= 8 issues, its sem
            # pool); everything later avoids scalar.
            queues = [[nc.sync, nc.scalar, nc.gpsimd]]
            qi = [0]

            def load(out, in_):
                qs = queues[0]
                qs[qi[0] % len(qs)].dma_start(out=out, in_=in_)
                qi[0] += 1

            def load_tT(b, it):
                t0 = it * TT
                tile_ = workp.tile([128, NC8, TT], f32r, tag="tT")
                for dc in range(NC8):
                    load(tile_[:, dc, :],
                         tT_d[b, dc * 128:(dc + 1) * 128, t0:t0 + TT])
                return tile_

            # dummy-matmul scratch: memset SBUF tile + PSUM slots cycled
            # from the psT pool. Dummies have no data deps, so the in-order
            # tensor queue runs them while real matmuls wait on DMA sems --
            # keeping the PE clock ramped. Each block writes one scratch
            # tile (WAW on the same engine needs no semaphores).
            dummy_sb = constp.tile([128, 128], bf16, tag="dummy_sb")
            nc.vector.memset(dummy_sb[:], 0.0)

            def dummies(n):
                dps = psT_p.tile([128, 128], f32, tag="tp")
                for _ in range(n):
                    nc.tensor.matmul(dps[:], dummy_sb[:], dummy_sb[:],
                                     start=True, stop=True)

            dummies(W0_N)

            wt_sb = constp.tile([128, NC8, D], f32r, tag="wt")
            wb_sb = constp.tile([128, NC8], f32, tag="wb")
            tT_first = workp.tile([128, NC8, TT], f32r, tag="tT")
            # the first matmul needs only wt[:, 0, 0:128] + tT(0,0) chunk 0:
            # both go on sync, the earliest-spinning DMA queue
            nc.sync.dma_start(out=wt_sb[:, 0, 0:128], in_=wT_d[0:128, 0:128])
            nc.sync.dma_start(out=tT_first[:, 0, :], in_=tT_d[0, 0:128, 0:TT])
            qi[0] = 2  # keep round-robin balanced past the two sync loads
            load(wt_sb[:, 0, 128:D], wT_d[0:128, 128:D])
            for dc in range(1, NC8):
                load(wt_sb[:, dc, :], wT_d[dc * 128:(dc + 1) * 128, :])
                load(tT_first[:, dc, :], tT_d[0, dc * 128:(dc + 1) * 128, 0:TT])
            load(wb_sb[:], wb_d[:])
            # scalar has now issued <=8 DMAs (its sem pool) -- no wrap waits.
            # Free it for ACT compute; all later DMA goes to sync+gpsimd.
            queues[0] = [nc.sync, nc.gpsimd]
            qi[0] = 0

            lT_tiles = {}
            lN_tiles = {}

            def load_locals(b):
                # lT keeps full 4KB-row DMAs (f32); lN rides bf16 (2KB rows,
                # half the bytes) since phase C runs bf16 anyway
                lT_sb = resp.tile([128, NC8, L], f32r, tag="lT")
                lN_sb = resp.tile([128, NC8, D], bf16, tag="lN")
                for c in range(NC8):
                    load(lT_sb[:, c, :], lT_d[b, c * 128:(c + 1) * 128, :])
                for c in range(NC8):
                    load(lN_sb[:, c, :], lN_d[b, c * 128:(c + 1) * 128, :])
                lT_tiles[b] = lT_sb
                lN_tiles[b] = lN_sb

            # startup DMA order: wt+tT(0,0) [6MB] -> lT(0) [4MB, gates the
            # first softmax] -> lN(0) [2MB bf16, gates phase C] -> tT(0,1).
            # Each engine's DMA sem pool is 8 deep: issue #k stalls until
            # issue #k-8's sem retires at its first consumer. lN(0) must sit
            # <=8 issues after early-retiring sems (startup/lT0), NOT after
            # tT(0,1) whose consumer A(0,1) is deferred to the prefetch slot.
            lT_b0s = resp.tile([128, NC8, L], f32r, tag="lT")
            for c in range(NC8):
                load(lT_b0s[:, c, :], lT_d[0, c * 128:(c + 1) * 128, :])
            lT_tiles[0] = lT_b0s
            lN_b0 = resp.tile([128, NC8, D], bf16, tag="lN")
            for c in range(NC8):
                load(lN_b0[:, c, :], lN_d[0, c * 128:(c + 1) * 128, :])
            lN_tiles[0] = lN_b0
            tT_b01 = load_tT(0, 1)
            # Sem-pool pads: lN(0) and tT(0,1) retire late (their first
            # consumers, C(0,0) and A(0,1), run at ~85us). 8 tiny loads per
            # ring put the next real loads >8 slots past them, so the
            # tT(1,0)/lT(1) wave issues as soon as bandwidth frees instead
            # of blocking on the pool wrap until ~85us.
            pad_sb = constp.tile([128, PAD_N], f32, tag="pad_sb")
            for p in range(PAD_N):
                load(pad_sb[:, p:p + 1], wb_d[:, 0:1])
            # identity for the diag(1/s) transpose trick -- not needed until
            # ~35us; built after the startup loads so gpsimd's DMA queue
            # isn't delayed
            identf = constp.tile([128, 128], f32, tag="identf")
            make_identity(nc, identf[:])
            ident_b = constp.tile([128, 128], bf16, tag="ident")
            nc.vector.tensor_copy(ident_b[:], identf[:])

            def phase_a(tT_sb, trickle=False):
                projT = projp.tile([128, NC8, TT], f32r, tag="projT")
                for ec in range(NC8):
                    psA = psMM_p.tile([128, TT], f32, tag="mm")
                    for dc in range(NC8):
                        nc.tensor.matmul(
                            psA[:],
                            wt_sb[:, dc, ec * 128:(ec + 1) * 128],
                            tT_sb[:, dc, :],
                            start=(dc == 0), stop=(dc == NC8 - 1))
                        # A(0,0) ec=0 consumes (wt,tT) chunk pairs as they
                        # land ~2.1us apart; filler keeps the clock up
                        if trickle and ec == 0 and dc < NC8 - 1:
                            dummies(DA_N)
                    nc.scalar.activation(projT[:, ec, :], psA[:], Act.Identity,
                                         bias=wb_sb[:, ec:ec + 1], scale=1.0)
                return projT

            def transposes(attnT, et, q, diag):
                # attnT[l,t] = et.T @ diag(1/s): one bf16 matmul per 128x128
                # block transposes AND applies the softmax scale
                for lq in range(NC8):
                    psT = psT_p.tile([128, 128], f32, tag="tp")
                    nc.tensor.matmul(psT[:],
                                     et[:, lq * 128:(lq + 1) * 128],
                                     diag[:], start=True, stop=True)
                    dst = attnT[:, lq, q * 128:(q + 1) * 128]
                    if lq % 2 == 0:
                        nc.vector.tensor_copy(dst, psT[:])
                    else:
                        nc.scalar.copy(dst, psT[:])

            projTs = {(0, 0): phase_a(tT_first, trickle=True)}
            preloaded = {(0, 1): tT_b01}
            dummies(W1_N)  # bridge: A(0,0) done ~22us, lT(0) lands ~31us

            tiles = [(b, it) for b in range(NB) for it in range(NT)]
            for i, (b, it) in enumerate(tiles):
                t0 = it * TT
                last = i == len(tiles) - 1
                if b > 0 and it == 0:
                    load_locals(b)
                projT = projTs[(b, it)]
                lT_sb, lN_sb = lT_tiles[b], lN_tiles[b]
                # ---- phase B + softmax; transposes one q behind; the last
                # tile also interleaves phase C per q so out-DMA starts early
                attnT = singlep.tile([128, NC8, TT], bf16, tag="attnT")

                def phase_c_half(h):
                    # last-tile phase C for a 256-t half: free dim 256 keeps
                    # the 97ns LDWEIGHTS hidden (128-free matmuls can't),
                    # while letting out-DMA start after T(q1) / T(q3)
                    for dc in range(NC8):
                        psC = psMM_p.tile([128, 256], f32, tag="mm")
                        for lq in range(NC8):
                            nc.tensor.matmul(
                                psC[:],
                                lN_sb[:, lq, dc * 128:(dc + 1) * 128],
                                attnT[:, lq, h * 256:(h + 1) * 256],
                                start=(lq == 0), stop=(lq == NC8 - 1))
                        oc = workp.tile([128, 256], f32, tag="outcq",
                                        bufs=4)
                        if dc % 2 == 0:
                            nc.vector.tensor_copy(oc[:], psC[:])
                        else:
                            nc.scalar.copy(oc[:], psC[:])
                        sq = [nc.scalar, nc.sync, nc.gpsimd][(h * NC8 + dc) % 3]
                        sq.dma_start(
                            out=outT_d[b, dc * 128:(dc + 1) * 128,
                                       t0 + h * 256:t0 + (h + 1) * 256],
                            in_=oc[:])

                pending = None
                for q in range(NQ):
                    psS = psS_p.tile([128, L], f32, tag="scores")
                    for lh in range(L // 512):
                        for ec in range(NC8):
                            nc.tensor.matmul(
                                psS[:, lh * 512:(lh + 1) * 512],
                                projT[:, ec, q * 128:(q + 1) * 128],
                                lT_sb[:, ec, lh * 512:(lh + 1) * 512],
                                start=(ec == 0), stop=(ec == NC8 - 1))
                            # B(0,0) q0 lh0 consumes lT(0) chunks as they
                            # land; filler keeps the clock up
                            if i == 0 and q == 0 and lh == 0 and ec < NC8 - 1:
                                dummies(DB_N)
                    nm = statsp.tile([128, 1], f32, tag="nm")
                    nc.vector.tensor_reduce(nm[:], psS[:],
                                            axis=mybir.AxisListType.X,
                                            op=mybir.AluOpType.max,
                                            negate=True)
                    et = workp.tile([128, L], bf16, tag="et")
                    s = statsp.tile([128, 1], f32, tag="s")
                    nc.scalar.activation(et[:], psS[:], Act.Exp,
                                         bias=nm[:, 0:1], scale=1.0,
                                         accum_out=s[:])
                    rr = statsp.tile([128, 1], f32, tag="rr")
                    nc.vector.reciprocal(rr[:], s[:])
                    diag = statsp.tile([128, 128], bf16, tag="diag", bufs=2)
                    nc.vector.tensor_scalar_mul(diag[:], ident_b[:],
                                                rr[:, 0:1])
                    if pending is not None:
                        transposes(attnT, *pending)
                        if last and pending[1] == 1:
                            phase_c_half(0)
                    pending = (et, q, diag)
                # prefetch the next tile's A phase here: its matmuls fill
                # the exp(q3)->transpose latency bubble and the batch
                # boundary, instead of the PE idling on them
                if i + 1 < len(tiles):
                    nb_, nit_ = tiles[i + 1]
                    if (nb_, nit_) not in projTs:
                        tT_next = preloaded.pop((nb_, nit_), None)
                        if tT_next is None:
                            tT_next = load_tT(nb_, nit_)
                        projTs[(nb_, nit_)] = phase_a(tT_next)
                transposes(attnT, *pending)
                if last:
                    phase_c_half(1)
                    continue
                # ---- phase C (monolithic, free dim 512): outT[d, t] ----
                for dc in range(NC8):
                    psC = psMM_p.tile([128, TT], f32, tag="mm")
                    for lq in range(NC8):
                        nc.tensor.matmul(
                            psC[:],
                            lN_sb[:, lq, dc * 128:(dc + 1) * 128],
                            attnT[:, lq, :],
                            start=(lq == 0), stop=(lq == NC8 - 1))
                    outcp = workp.tile([128, TT], f32, tag="outcp")
                    if dc % 2 == 0:
                        nc.vector.tensor_copy(outcp[:], psC[:])
                    else:
                        nc.scalar.copy(outcp[:], psC[:])
                    # out-DMAs all ride scalar: an out's sem-pool slot frees
                    # only when its (late-executing) transfer completes, so
                    # parking outs on the sync/gpsimd load rings would wedge
                    # the next tiles' prefetch loads behind them (8-deep pool)
                    nc.scalar.dma_start(
                        out=outT_d[b, dc * 128:(dc + 1) * 128, t0:t0 + TT],
                        in_=outcp[:])
    nc.compile()
    return nc


def _get_nc():
    if "nc" not in _cache:
        _cache["nc"] = _build()
    return _cache["nc"]


def _prep_inputs(text_features, local_features, W_w, W_b):
    import ml_dtypes

    text = np.asarray(text_features, dtype=np.float32)
    local = np.asarray(local_features, dtype=np.float32)
    W = np.asarray(W_w, dtype=np.float32)
    bvec = np.asarray(W_b, dtype=np.float32)

    wT = np.ascontiguousarray(W.T)                       # [d, e]
    wb = np.ascontiguousarray(bvec.reshape(NC8, 128).T)  # [128, ec]
    in_maps = []
    for c in range(NCORES):
        sl = slice(c * NB, (c + 1) * NB)
        in_maps.append({
            "tT": np.ascontiguousarray(text[sl].transpose(0, 2, 1)),
            "lT": np.ascontiguousarray(local[sl].transpose(0, 2, 1)),
            "lN": np.ascontiguousarray(local[sl].astype(ml_dtypes.bfloat16)),
            "wT": wT,
            "wb": wb,
        })
    return in_maps


def _run(inputs, trace=False):
    from concourse.bass_utils import run_bass_kernel_spmd

    nc = _get_nc()
    in_maps = _prep_inputs(**inputs)
    res = run_bass_kernel_spmd(nc, in_maps, list(range(NCORES)), trace=trace)
    out = np.empty((B, T, D), dtype=np.float32)
    for c in range(NCORES):
        outT = res.results[c]["outT"]                    # [NB, d, t]
        out[c * NB:(c + 1) * NB] = outT.transpose(0, 2, 1)
    return out, res


def kernel(**inputs):
    out, _ = _run(inputs, trace=False)
    return out


# revision 19
# speedup vs baseline: 1.0536x; 1.0241x over previous
"""Trainium2 Bass kernel for visual cross-attention:
    proj   = text @ W_w.T + W_b          [B,T,D]
    scores = proj @ local.T              [B,T,L]
    attn   = softmax(scores, axis=-1)
    out    = attn @ local                [B,T,D]

B=16, T=L=D=1024, fp32. Data-parallel over batch: 8 cores x 2 batches.
Score-path matmuls run as float32r (full PE rate at free>=256, ~1e-3 rel
err); the attention-weight path (attnT, lN, phase C) runs bf16 -- the
softmax output is in [0,1] so bf16 keeps ~3e-3 overall rel err, halves
the lN DMA, and lets the attn transposes run at 1.0 cycles/row.

Per core, per batch, per T-tile (512 t's):
  A: projT[e,t]   = W^T-chunks.T @ textT-chunks        (PE, accum over d)
  B: scores[t,l]  = projT-chunks.T @ localT-chunks     (PE, accum over e)
     softmax over l (free dim): DVE max, ACT exp(+bias,-max, accum sum)
  T: attnT[l,t]   = et-chunk.T @ diag(1/s)  -- ONE bf16 matmul per
     128x128 block fuses the transpose with the softmax 1/sum scale
     (diag = ident * recip(s), built on DVE). Emitted one q-chunk behind
     B so the next B covers softmax latency.
  C: outT[d,t]    = lN-chunks.T @ attnT-chunks         (PE bf16, accum l)

PE p-state: the tensor engine clocks 0.65/1.2 GHz until ~3us of
continuous execution, 2.4 GHz after. The DMA-bound startup would leave
it cold, so dummy bf16 matmuls (on a memset tile, into a scratch PSUM
slot) pad the gaps: a t=0 warmup train, fills between phase-A chunk
arrivals, and a bridge to the first phase B. The last tile interleaves
T/C per q-block so the output DMA starts ~10us earlier.
Host side only reshapes/transposes (layout prep + final [d,t]->[t,d]).
"""
import sys

sys.path.insert(0, "/opt/trn_rl_repo")
import numpy as np

B, T, L, D = 16, 1024, 1024, 1024
NCORES = 8
NB = B // NCORES          # batches per core
TT = 512                  # T-tile (moving dim for phases A/C)
NT = T // TT              # T-tiles per batch
NC8 = D // 128            # 128-chunks along d/e/l
NQ = TT // 128            # 128-t chunks per T-tile

# PE warmup / gap-filler dummy matmul counts (tuned on trace evidence)
W0_N = 64                 # t=0 warmup train
DA_N = 28                 # per-dc filler inside A(0,0) ec=0 trickle
W1_N = 40                 # bridge A(0,0) -> B(0,0)
DB_N = 10                 # per-ec filler inside B(0,0) q0 lh0 trickle

_cache = {}


def _build():
    import concourse.tile as tile
    from concourse import bacc, mybir
    from concourse.masks import make_identity

    f32 = mybir.dt.float32
    f32r = mybir.dt.float32r
    bf16 = mybir.dt.bfloat16
    Act = mybir.ActivationFunctionType

    nc = bacc.Bacc("TRN2", target_bir_lowering=False, debug=False,
                   num_devices=NCORES)
    tT_d = nc.dram_tensor("tT", [NB, D, T], f32r, kind="ExternalInput").ap()
    lT_d = nc.dram_tensor("lT", [NB, D, L], f32r, kind="ExternalInput").ap()
    lN_d = nc.dram_tensor("lN", [NB, L, D], bf16, kind="ExternalInput").ap()
    wT_d = nc.dram_tensor("wT", [D, D], f32r, kind="ExternalInput").ap()
    wb_d = nc.dram_tensor("wb", [128, NC8], f32, kind="ExternalInput").ap()
    outT_d = nc.dram_tensor("outT", [NB, D, T], f32, kind="ExternalOutput").ap()

    with tile.TileContext(nc) as tc:
        with tc.tile_pool(name="const", bufs=1) as constp, \
             tc.tile_pool(name="res", bufs=1) as resp, \
             tc.tile_pool(name="work", bufs=2) as workp, \
             tc.tile_pool(name="proj", bufs=3) as projp, \
             tc.tile_pool(name="single", bufs=1) as singlep, \
             tc.tile_pool(name="stats", bufs=8) as statsp, \
             tc.tile_pool(name="psS", bufs=2, space="PSUM") as psS_p, \
             tc.tile_pool(name="psMM", bufs=2, space="PSUM") as psMM_p, \
             tc.tile_pool(name="psT", bufs=2, space="PSUM") as psT_p:

            # round-robin loads across all 3 DMA-capable queues (sync/scalar
            # HWDGE + gpsimd SWDGE), in consumption order; each queue peaks
            # ~110-130GB/s, together ~350GB/s (HBM-bound).
            # The scalar engine is a DMA-issue engine (qAct HWDGE) AND the
            # softmax/copy engine. A long load backlog on it blocks ACT
            # compute behind DMA sem-pool wrap waits. So: the startup-
            # critical prefix (wt, tT(0,*), lT(0)) round-robins over all 3
            # queues for max bandwidth (scalar kept <= 8 issues, its sem
            # pool); everything later avoids scalar.
            queues = [[nc.sync, nc.scalar, nc.gpsimd]]
            qi = [0]

            def load(out, in_):
                qs = queues[0]
                qs[qi[0] % len(qs)].dma_start(out=out, in_=in_)
                qi[0] += 1

            def load_tT(b, it):
                t0 = it * TT
                tile_ = workp.tile([128, NC8, TT], f32r, tag="tT")
                for dc in range(NC8):
                    load(tile_[:, dc, :],
                         tT_d[b, dc * 128:(dc + 1) * 128, t0:t0 + TT])
                return tile_

            # dummy-matmul scratch: memset SBUF tile + PSUM slots cycled
            # from the psT pool. Dummies have no data deps, so the in-order
            # tensor queue runs them while real matmuls wait on DMA sems --
            # keeping the PE clock ramped. Each block writes one scratch
            # tile (WAW on the same engine needs no semaphores).
            dummy_sb = constp.tile([128, 128], bf16, tag="dummy_sb")
            nc.vector.memset(dummy_sb[:], 0.0)

            def dummies(n):
                dps = psT_p.tile([128, 128], f32, tag="tp")
                for _ in range(n):
                    nc.tensor.matmul(dps[:], dummy_sb[:], dummy_sb[:],
                                     start=True, stop=True)

            dummies(W0_N)

            wt_sb = constp.tile([128, NC8, D], f32r, tag="wt")
            wb_sb = constp.tile([128, NC8], f32, tag="wb")
            tT_first = workp.tile([128, NC8, TT], f32r, tag="tT")
            # the first matmul needs only wt[:, 0, 0:128] + tT(0,0) chunk 0:
            # both go on sync, the earliest-spinning DMA queue
            nc.sync.dma_start(out=wt_sb[:, 0, 0:128], in_=wT_d[0:128, 0:128])
            nc.sync.dma_start(out=tT_first[:, 0, :], in_=tT_d[0, 0:128, 0:TT])
            qi[0] = 2  # keep round-robin balanced past the two sync loads
            load(wt_sb[:, 0, 128:D], wT_d[0:128, 128:D])
            for dc in range(1, NC8):
                load(wt_sb[:, dc, :], wT_d[dc * 128:(dc + 1) * 128, :])
                load(tT_first[:, dc, :], tT_d[0, dc * 128:(dc + 1) * 128, 0:TT])
            load(wb_sb[:], wb_d[:])
            # scalar has now issued <=8 DMAs (its sem pool) -- no wrap waits.
            # Free it for ACT compute; all later DMA goes to sync+gpsimd.
            queues[0] = [nc.sync, nc.gpsimd]
            qi[0] = 0

            lT_tiles = {}
            lN_tiles = {}

            def load_locals(b):
                # lT keeps full 4KB-row DMAs (f32); lN rides bf16 (2KB rows,
                # half the bytes) since phase C runs bf16 anyway
                lT_sb = resp.tile([128, NC8, L], f32r, tag="lT")
                lN_sb = resp.tile([128, NC8, D], bf16, tag="lN")
                for c in range(NC8):
                    load(lT_sb[:, c, :], lT_d[b, c * 128:(c + 1) * 128, :])
                for c in range(NC8):
                    load(lN_sb[:, c, :], lN_d[b, c * 128:(c + 1) * 128, :])
                lT_tiles[b] = lT_sb
                lN_tiles[b] = lN_sb

            # startup DMA order: wt+tT(0,0) [6MB] -> lT(0) [4MB, gates the
            # first softmax] -> lN(0) [2MB bf16, gates phase C] -> tT(0,1).
            # Each engine's DMA sem pool is 8 deep: issue #k stalls until
            # issue #k-8's sem retires at its first consumer. lN(0) must sit
            # <=8 issues after early-retiring sems (startup/lT0), NOT after
            # tT(0,1) whose consumer A(0,1) is deferred to the prefetch slot.
            lT_b0s = resp.tile([128, NC8, L], f32r, tag="lT")
            for c in range(NC8):
                load(lT_b0s[:, c, :], lT_d[0, c * 128:(c + 1) * 128, :])
            lT_tiles[0] = lT_b0s
            lN_b0 = resp.tile([128, NC8, D], bf16, tag="lN")
            for c in range(NC8):
                load(lN_b0[:, c, :], lN_d[0, c * 128:(c + 1) * 128, :])
            lN_tiles[0] = lN_b0
            tT_b01 = load_tT(0, 1)
            # identity for the diag(1/s) transpose trick -- not needed until
            # ~35us; built after the startup loads so gpsimd's DMA queue
            # isn't delayed
            identf = constp.tile([128, 128], f32, tag="identf")
            make_identity(nc, identf[:])
            ident_b = constp.tile([128, 128], bf16, tag="ident")
            nc.vector.tensor_copy(ident_b[:], identf[:])

            def phase_a(tT_sb, trickle=False):
                projT = projp.tile([128, NC8, TT], f32r, tag="projT")
                for ec in range(NC8):
                    psA = psMM_p.tile([128, TT], f32, tag="mm")
                    for dc in range(NC8):
                        nc.tensor.matmul(
                            psA[:],
                            wt_sb[:, dc, ec * 128:(ec + 1) * 128],
                            tT_sb[:, dc, :],
                            start=(dc == 0), stop=(dc == NC8 - 1))
                        # A(0,0) ec=0 consumes (wt,tT) chunk pairs as they
                        # land ~2.1us apart; filler keeps the clock up
                        if trickle and ec == 0 and dc < NC8 - 1:
                            dummies(DA_N)
                    nc.scalar.activation(projT[:, ec, :], psA[:], Act.Identity,
                                         bias=wb_sb[:, ec:ec + 1], scale=1.0)
                return projT

            def transposes(attnT, et, q, diag):
                # attnT[l,t] = et.T @ diag(1/s): one bf16 matmul per 128x128
                # block transposes AND applies the softmax scale
                for lq in range(NC8):
                    psT = psT_p.tile([128, 128], f32, tag="tp")
                    nc.tensor.matmul(psT[:],
                                     et[:, lq * 128:(lq + 1) * 128],
                                     diag[:], start=True, stop=True)
                    dst = attnT[:, lq, q * 128:(q + 1) * 128]
                    if lq % 2 == 0:
                        nc.vector.tensor_copy(dst, psT[:])
                    else:
                        nc.scalar.copy(dst, psT[:])

            projTs = {(0, 0): phase_a(tT_first, trickle=True)}
            preloaded = {(0, 1): tT_b01}
            dummies(W1_N)  # bridge: A(0,0) done ~22us, lT(0) lands ~31us

            tiles = [(b, it) for b in range(NB) for it in range(NT)]
            for i, (b, it) in enumerate(tiles):
                t0 = it * TT
                last = i == len(tiles) - 1
                if b > 0 and it == 0:
                    load_locals(b)
                projT = projTs[(b, it)]
                lT_sb, lN_sb = lT_tiles[b], lN_tiles[b]
                # ---- phase B + softmax; transposes one q behind; the last
                # tile also interleaves phase C per q so out-DMA starts early
                attnT = singlep.tile([128, NC8, TT], bf16, tag="attnT")

                def phase_c_half(h):
                    # last-tile phase C for a 256-t half: free dim 256 keeps
                    # the 97ns LDWEIGHTS hidden (128-free matmuls can't),
                    # while letting out-DMA start after T(q1) / T(q3)
                    for dc in range(NC8):
                        psC = psMM_p.tile([128, 256], f32, tag="mm")
                        for lq in range(NC8):
                            nc.tensor.matmul(
                                psC[:],
                                lN_sb[:, lq, dc * 128:(dc + 1) * 128],
                                attnT[:, lq, h * 256:(h + 1) * 256],
                                start=(lq == 0), stop=(lq == NC8 - 1))
                        oc = workp.tile([128, 256], f32, tag="outcq",
                                        bufs=4)
                        if dc % 2 == 0:
                            nc.vector.tensor_copy(oc[:], psC[:])
                        else:
                            nc.scalar.copy(oc[:], psC[:])
                        sq = [nc.scalar, nc.sync, nc.gpsimd][(h * NC8 + dc) % 3]
                        sq.dma_start(
                            out=outT_d[b, dc * 128:(dc + 1) * 128,
                                       t0 + h * 256:t0 + (h + 1) * 256],
                            in_=oc[:])

                pending = None
                for q in range(NQ):
                    psS = psS_p.tile([128, L], f32, tag="scores")
                    for lh in range(L // 512):
                        for ec in range(NC8):
                            nc.tensor.matmul(
                                psS[:, lh * 512:(lh + 1) * 512],
                                projT[:, ec, q * 128:(q + 1) * 128],
                                lT_sb[:, ec, lh * 512:(lh + 1) * 512],
                                start=(ec == 0), stop=(ec == NC8 - 1))
                            # B(0,0) q0 lh0 consumes lT(0) chunks as they
                            # land; filler keeps the clock up
                            if i == 0 and q == 0 and lh == 0 and ec < NC8 - 1:
                                dummies(DB_N)
                    nm = statsp.tile([128, 1], f32, tag="nm")
                    nc.vector.tensor_reduce(nm[:], psS[:],
                                            axis=mybir.AxisListType.X,
                                            op=mybir.AluOpType.max,
                                            negate=True)
                    et = workp.tile([128, L], bf16, tag="et")
                    s = statsp.tile([128, 1], f32, tag="s")
                    nc.scalar.activation(et[:], psS[:], Act.Exp,
                                         bias=nm[:, 0:1], scale=1.0,
                                         accum_out=s[:])
                    rr = statsp.tile([128, 1], f32, tag="rr")
                    nc.vector.reciprocal(rr[:], s[:])
                    diag = statsp.tile([128, 128], bf16, tag="diag", bufs=2)
                    nc.vector.tensor_scalar_mul(diag[:], ident_b[:],
                                                rr[:, 0:1])
                    if pending is not None:
                        transposes(attnT, *pending)
                        if last and pending[1] == 2:
                            phase_c_half(0)
                    if i == 0 and q == 2:
                        # Early sem retirement: lN(0)/tT(0,1) are first read
                        # by C(0,0)/A(0,1) at ~85us, wedging the 8-deep DMA
                        # sem pools (later loads wait on their retirement).
                        # Two strided 1-column matmuls make the PE execute
                        # the waits here (~63us; the data lands ~55us), so
                        # the tT(1,0)/lT(1) wave issues while bandwidth is
                        # free instead of fighting C(0,0)'s out-DMAs.
                        psE = psT_p.tile([128, NC8], f32, tag="tp")
                        nc.tensor.matmul(psE[:], dummy_sb[:],
                                         lN_b0[:, 0:NC8, 0:1],
                                         start=True, stop=True)
                        psE2 = psT_p.tile([128, NC8], f32, tag="tp")
                        nc.tensor.matmul(psE2[:], wt_sb[:, 0, 0:128],
                                         tT_b01[:, 0:NC8, 0:1],
                                         start=True, stop=True)
                    pending = (et, q, diag)
                # prefetch the next tile's A phase here: its matmuls fill
                # the exp(q3)->transpose latency bubble and the batch
                # boundary, instead of the PE idling on them
                if i + 1 < len(tiles):
                    nb_, nit_ = tiles[i + 1]
                    if (nb_, nit_) not in projTs:
                        tT_next = preloaded.pop((nb_, nit_), None)
                        if tT_next is None:
                            tT_next = load_tT(nb_, nit_)
                        projTs[(nb_, nit_)] = phase_a(tT_next)
                transposes(attnT, *pending)
                if last:
                    phase_c_half(1)
                    continue
                # ---- phase C (monolithic, free dim 512): outT[d, t] ----
                for dc in range(NC8):
                    psC = psMM_p.tile([128, TT], f32, tag="mm")
                    for lq in range(NC8):
                        nc.tensor.matmul(
                            psC[:],
                            lN_sb[:, lq, dc * 128:(dc + 1) * 128],
                            attnT[:, lq, :],
                            start=(lq == 0), stop=(lq == NC8 - 1))
                    outcp = workp.tile([128, TT], f32, tag="outcp",
                                       bufs=4)
                    if dc % 2 == 0:
                        nc.vector.tensor_copy(outcp[:], psC[:])
                    else:
                        nc.scalar.copy(outcp[:], psC[:])
                    # out-DMAs all ride scalar: an out's sem-pool slot frees
                    # only when its (late-executing) transfer completes, so
                    # parking outs on the sync/gpsimd load rings would wedge
                    # the next tiles' prefetch loads behind them (8-deep pool)
                    nc.scalar.dma_start(
                        out=outT_d[b, dc * 128:(dc + 1) * 128, t0:t0 + TT],
                        in_=outcp[:])
    nc.compile()
    return nc


def _get_nc():
    if "nc" not in _cache:
        _cache["nc"] = _build()
    return _cache["nc"]


def _prep_inputs(text_features, local_features, W_w, W_b):
    import ml_dtypes

    text = np.asarray(text_features, dtype=np.float32)
    local = np.asarray(local_features, dtype=np.float32)
    W = np.asarray(W_w, dtype=np.float32)
    bvec = np.asarray(W_b, dtype=np.float32)

    wT = np.ascontiguousarray(W.T)                       # [d, e]
    wb = np.ascontiguousarray(bvec.reshape(NC8, 128).T)  # [128, ec]
    in_maps = []
    for c in range(NCORES):
        sl = slice(c * NB, (c + 1) * NB)
        in_maps.append({
            "tT": np.ascontiguousarray(text[sl].transpose(0, 2, 1)),
            "lT": np.ascontiguousarray(local[sl].transpose(0, 2, 1)),
            "lN": np.ascontiguousarray(local[sl].astype(ml_dtypes.bfloat16)),
            "wT": wT,
            "wb": wb,
        })
    return in_maps


def _run(inputs, trace=False):
    from concourse.bass_utils import run_bass_kernel_spmd

    nc = _get_nc()
    in_maps = _prep_inputs(**inputs)
    res = run_bass_kernel_spmd(nc, in_maps, list(range(NCORES)), trace=trace)
    out = np.empty((B, T, D), dtype=np.float32)
    for c in range(NCORES):
        outT = res.results[c]["outT"]                    # [NB, d, t]
        out[c * NB:(c + 1) * NB] = outT.transpose(0, 2, 1)
    return out, res


def kernel(**inputs):
    out, _ = _run(inputs, trace=False)
    return out
